# revision 1
# baseline (speedup 1.0000x reference)
"""Expert-parallel MoE FFN kernel for 8 trn2 NeuronCores.

Problem (per full input):
  x [4, 8, 512, 1024], audio_feat [4, 256, 1024],
  W1/Wa [8, 1024, 4096], b1 [8, 4096], W2 [8, 4096, 1024], b2 [8, 1024]
  out[b,e,n,:] = gelu_tanh(x[b,e,n] @ W1[e] + b1[e] + mean(audio_feat[b]) @ Wa[e]) @ W2[e] + b2[e]

Sharding: expert-parallel — core e owns expert e (weights + x[:, e] slice);
audio_feat replicated. No collectives needed: shard/gather on host.

Per-core kernel (all matmuls in float32r: fp32 with 11-bit mantissa,
full PE rate):
  - audio pooling via matmul with a block-indicator matrix (one-hot / 256)
  - GEMM1 produces h^T tiles [dff, tok] so GEMM2 can consume them as
    stationary operands without transposes (x passed pre-transposed)
  - dff is blocked (2x256 + 7x512 on the first token half, 8x512 on the
    second); GEMM2 partials accumulate into an SBUF accumulator via DVE
    adds; tokens processed in 2 halves of 1024 so everything fits in SBUF.
"""
from contextlib import ExitStack

import ml_dtypes
import numpy as np

import concourse.bass as bass
import concourse.tile as tile
from concourse import bacc, mybir
from concourse.bass_utils import run_bass_kernel_spmd

F32 = mybir.dt.float32
F32R = mybir.dt.float32r
BF16 = mybir.dt.bfloat16
AF = mybir.ActivationFunctionType

B, E, N, D = 4, 8, 512, 1024
DFF = 4 * D
NA = 256
TOK = B * N            # 2048 tokens per expert
KC = D // 128          # 8 d-chunks
NHALF = 2              # token halves
TOKH = TOK // NHALF    # 1024
NDFB = 8               # dff blocks
DFB = DFF // NDFB      # 512
NC_CORES = 8

_cache = {}


def _build():
    nc = bacc.Bacc("TRN2", target_bir_lowering=False, debug=False,
                   num_devices=NC_CORES)

    xT_d = nc.declare_dram_parameter("xT", [NHALF, 2, 128, KC, N], F32R, isOutput=False)
    af_d = nc.declare_dram_parameter("af", [2, 128, KC // 2, D], BF16, isOutput=False)
    ind_d = nc.declare_dram_parameter("ind", [128, KC, B], BF16, isOutput=False)
    id4_d = nc.declare_dram_parameter("id4", [B, B], F32, isOutput=False)
    w1_d = nc.declare_dram_parameter("w1", [D, DFF], F32R, isOutput=False)
    wa_d = nc.declare_dram_parameter("wa", [D, DFF], F32R, isOutput=False)
    w2_d = nc.declare_dram_parameter("w2", [DFF, D], F32R, isOutput=False)
    b1t_d = nc.declare_dram_parameter("b1t", [128, DFF // 128], F32, isOutput=False)
    b2b_d = nc.declare_dram_parameter("b2b", [128, D], F32, isOutput=False)
    out_d = nc.declare_dram_parameter("out", [TOK, D], F32, isOutput=True)

    with tile.TileContext(nc) as tc, ExitStack() as ctx:
        sb = ctx.enter_context(tc.tile_pool(name="sb", bufs=1))
        ps = ctx.enter_context(
            tc.tile_pool(name="ps", bufs=1, space=bass.MemorySpace.PSUM))

        # ---- small persistent tiles -------------------------------------
        ind_t = sb.tile([128, KC, B], BF16, name="ind_t")
        id4_t = sb.tile([B, B], F32, name="id4_t")
        b1t_t = sb.tile([128, DFF // 128], F32, name="b1t_t")
        b2b_t = sb.tile([128, D], F32, name="b2b_t")
        apT_t = sb.tile([128, KC, B], F32R, name="apT_t")
        baud_t = sb.tile([128, DFF // 128, B], F32, name="baud_t")
        nc.sync.dma_start(out=ind_t[:], in_=ind_d.ap())
        nc.sync.dma_start(out=id4_t[:], in_=id4_d.ap())
        nc.sync.dma_start(out=b1t_t[:], in_=b1t_d.ap())
        nc.sync.dma_start(out=b2b_t[:], in_=b2b_d.ap())

        # ---- DMA helpers ------------------------------------------------
        def dma_w1(half, boff, blen, eng=None):
            w1_t = sb.tile([128, KC, blen], F32R, name=f"w1_{half}_{boff}",
                           tag="w1s", bufs=2)
            (eng or nc.sync).dma_start(
                out=w1_t[:],
                in_=w1_d.ap()[:, boff:boff + blen]
                    .rearrange("(kc p) f -> p kc f", p=128))
            return w1_t

        def dma_w2(half, boff, blen, eng=None):
            w2_t = sb.tile([128, blen // 128, D], F32R,
                           name=f"w2_{half}_{boff}", tag="w2s", bufs=2)
            (eng or nc.sync).dma_start(
                out=w2_t[:],
                in_=w2_d.ap()[boff:boff + blen, :]
                    .rearrange("(c p) d -> p c d", p=128))
            return w2_t

        def dma_wa(boff, blen, eng=None):
            wa_t = sb.tile([128, KC, blen], F32R, name=f"wa_{boff}",
                           tag="was", bufs=1)
            (eng or nc.sync).dma_start(
                out=wa_t[:],
                in_=wa_d.ap()[:, boff:boff + blen]
                    .rearrange("(kc p) f -> p kc f", p=128))
            return wa_t

        def dma_xT(half, b, xT_t):
            nc.sync.dma_start(
                out=xT_t[:, :, b * N:(b + 1) * N],
                in_=xT_d.ap()[half, b])

        # ---- start-up: hand-ordered DMA queue ---------------------------
        # af tile shares the "xT" tag/slot (same shape); dead after phase A.
        # bufs=2 on the tag so xT half-1 can prefetch into af's slot.
        af_t = sb.tile([128, KC, D], BF16, name="af_t", tag="xT", bufs=2)
        for hc in range(2):
            nc.sync.dma_start(
                out=af_t[:, hc * (KC // 2):(hc + 1) * (KC // 2), :],
                in_=af_d.ap()[hc])
        BLK0 = 256
        xT0_t = sb.tile([128, KC, TOKH], F32R, name="xT_0", tag="xT", bufs=2)
        dma_xT(0, 0, xT0_t)
        w1_00 = dma_w1(0, 0, BLK0, eng=nc.scalar)
        wa_0 = dma_wa(0, BLK0, eng=nc.gpsimd)
        dma_xT(0, 1, xT0_t)
        w2_00 = dma_w2(0, 0, BLK0, eng=nc.scalar)

        # ---- PE warm-up -------------------------------------------------
        # The first ~18us are DMA-bound; keep the PE busy on throwaway
        # matmuls over the tiny ind tile so the HAM clock gate stays at
        # full rate when real work arrives.
        psW = ps.tile([B, B], F32, name="psW", tag="ps2b", bufs=2)
        for i in range(150):
            nc.tensor.matmul(psW[:], ind_t[:, 0, :], ind_t[:, 0, :],
                             start=True, stop=True)

        # ---- phase A: audio mean-pool -> apT [d-chunk, b] ---------------
        # pooled [4, d] = ind.T @ af  (stationary is the tiny [128,4]
        # indicator -> negligible weight-load time), then transpose.
        ap_sb = sb.tile([B, D], F32, name="ap_sb")
        for dh in range(2):
            psP = ps.tile([B, 512], F32, name=f"psP{dh}",
                          tag=f"ps1{'ab'[dh]}", bufs=2)
            for tc_ in range(KC):
                nc.tensor.matmul(
                    psP[:], ind_t[:, tc_, :],
                    af_t[:, tc_, dh * 512:(dh + 1) * 512],
                    start=(tc_ == 0), stop=(tc_ == KC - 1))
            nc.vector.tensor_copy(ap_sb[:, dh * 512:(dh + 1) * 512], psP[:])
        for dc in range(KC):
            psQ = ps.tile([128, B], F32, name=f"psQ{dc}", tag="ps2a", bufs=2)
            nc.tensor.transpose(
                psQ[:], ap_sb[:, dc * 128:(dc + 1) * 128], id4_t[:])
            nc.vector.tensor_copy(apT_t[:, dc, :], psQ[:])

        # ---- phase B: main loop -----------------------------------------
        for half in range(NHALF):
            if half == 0:
                xT_t = xT0_t
            else:
                xT_t = sb.tile([128, KC, TOKH], F32R, name=f"xT_{half}",
                               tag="xT", bufs=2)
                for b in range(2):
                    dma_xT(half, b, xT_t)
            oacc = [
                sb.tile([128, D], F32, name=f"oacc_{half}_{t}",
                        tag=f"oacc{t}", bufs=1)
                for t in range(TOKH // 128)
            ]
            if half == 0:
                blocks = [(0, BLK0), (BLK0, BLK0)] + [
                    (boff, DFB) for boff in range(2 * BLK0, DFF, DFB)]
            else:
                # reverse order: the first block reuses the w1/w2 tiles still
                # resident from half-0's last block (no DMA, no boundary wait)
                blocks = [(boff, DFB)
                          for boff in reversed(range(0, DFF, DFB))]
            for blk_i, (boff, blen) in enumerate(blocks):
                first = (half == 0 and boff == 0)
                first_blk = (blk_i == 0)
                last_blk = (blk_i == len(blocks) - 1)
                reuse = (half == 1 and blk_i == 0)
                ncc = blen // 128
                w1_t = (w1_00 if first else
                        (prev_w1 if reuse else dma_w1(half, boff, blen)))
                if half == 0:
                    wa_t = wa_0 if first else dma_wa(boff, blen)
                    # audio_h [4, blen] = apT.T @ wa_block (stationary is the
                    # tiny [128,4] apT chunk -> negligible weight-load time)
                    psH = ps.tile([B, blen], F32, name=f"psH{boff}",
                                  tag="ps1a", bufs=2)
                    for kc in range(KC):
                        nc.tensor.matmul(
                            psH[:], apT_t[:, kc, :], wa_t[:, kc, :],
                            start=(kc == 0), stop=(kc == KC - 1))
                    ah_t = sb.tile([B, blen], F32, name=f"ah_{boff}", tag="ah",
                                   bufs=2)
                    nc.vector.tensor_copy(ah_t[:], psH[:])
                    for c in range(ncc):
                        cg = boff // 128 + c
                        psT = ps.tile([128, B], F32, name=f"psT{cg}",
                                      tag="ps2a", bufs=2)
                        nc.tensor.transpose(
                            psT[:], ah_t[:, c * 128:(c + 1) * 128], id4_t[:])
                        nc.vector.tensor_scalar_add(
                            baud_t[:, cg, :], psT[:], b1t_t[:, cg:cg + 1])

                w2_t = (w2_00 if first else
                        (prev_w2 if reuse else dma_w2(half, boff, blen)))
                prev_w1, prev_w2 = w1_t, w2_t

                # GEMM1: h^T tiles [128 dff, 512 tok] for both b-blocks
                hT = []
                for c in range(ncc):
                    cg = boff // 128 + c
                    p1 = [
                        ps.tile([128, N], F32, name=f"ps1_{half}_{boff}_{c}_{b}",
                                tag=f"ps1{'ab'[b]}", bufs=2)
                        for b in range(2)
                    ]
                    for kc in range(KC):
                        for b in range(2):
                            nc.tensor.matmul(
                                p1[b][:], w1_t[:, kc, c * 128:(c + 1) * 128],
                                xT_t[:, kc, b * N:(b + 1) * N],
                                start=(kc == 0), stop=(kc == KC - 1))
                    row = []
                    for b in range(2):
                        bg = half * 2 + b
                        h = sb.tile([128, N], F32R,
                                    name=f"hT_{half}_{boff}_{c}_{b}",
                                    tag=f"hT{c}b{b}", bufs=1)
                        nc.scalar.activation(
                            h[:], p1[b][:], AF.Gelu_apprx_tanh,
                            bias=baud_t[:, cg, bg:bg + 1], scale=1.0)
                        row.append(h)
                    hT.append(row)

                # GEMM2: out tiles [128 tok, 512 d], accumulate over blocks
                for b in range(2):
                    for ts in range(N // 128):
                        tsg = b * (N // 128) + ts
                        p2 = [
                            ps.tile([128, 512], F32,
                                    name=f"ps2_{half}_{boff}_{tsg}_{dh}",
                                    tag=f"ps2{'ab'[dh]}", bufs=2)
                            for dh in range(2)
                        ]
                        for c in range(ncc):
                            for dh in range(2):
                                nc.tensor.matmul(
                                    p2[dh][:],
                                    hT[c][b][:, ts * 128:(ts + 1) * 128],
                                    w2_t[:, c, dh * 512:(dh + 1) * 512],
                                    start=(c == 0), stop=(c == ncc - 1))
                        for dh in range(2):
                            dst = oacc[tsg][:, dh * 512:(dh + 1) * 512]
                            if first_blk:
                                nc.vector.tensor_add(
                                    dst, p2[dh][:],
                                    b2b_t[:, dh * 512:(dh + 1) * 512])
                            else:
                                nc.vector.tensor_add(dst, dst, p2[dh][:])
                        if last_blk:
                            row0 = half * TOKH + tsg * 128
                            nc.sync.dma_start(
                                out=out_d.ap()[row0:row0 + 128, :],
                                in_=oacc[tsg][:])

    nc.compile()
    return nc


def _get_nc():
    if "nc" not in _cache:
        _cache["nc"] = _build()
    return _cache["nc"]


def kernel(x, audio_feat, W1, b1, Wa, W2, b2):
    x = np.asarray(x, dtype=np.float32)
    audio_feat = np.asarray(audio_feat, dtype=np.float32)
    W1 = np.asarray(W1, dtype=np.float32)
    b1 = np.asarray(b1, dtype=np.float32)
    Wa = np.asarray(Wa, dtype=np.float32)
    W2 = np.asarray(W2, dtype=np.float32)
    b2 = np.asarray(b2, dtype=np.float32)

    nc = _get_nc()

    af = np.ascontiguousarray(
        audio_feat.reshape(2, KC // 2, 128, D).transpose(0, 2, 1, 3)
    ).astype(ml_dtypes.bfloat16)
    # indicator: token t (= tc*128 + p) belongs to batch b = t // NA
    ind = np.zeros((128, KC, B), dtype=ml_dtypes.bfloat16)
    for tc_ in range(KC):
        ind[:, tc_, (tc_ * 128) // NA] = 1.0 / NA
    id4 = np.eye(B, dtype=np.float32)

    in_maps = []
    for e in range(E):
        xT = np.ascontiguousarray(
            x[:, e].reshape(TOK, D).T
            .reshape(KC, 128, NHALF, 2, N).transpose(2, 3, 1, 0, 4))
        in_maps.append({
            "xT": xT,
            "af": af,
            "ind": ind,
            "id4": id4,
            "w1": np.ascontiguousarray(W1[e]),
            "wa": np.ascontiguousarray(Wa[e]),
            "w2": np.ascontiguousarray(W2[e]),
            "b1t": np.ascontiguousarray(b1[e].reshape(DFF // 128, 128).T),
            "b2b": np.ascontiguousarray(np.broadcast_to(b2[e], (128, D))),
        })

    # A prior tenant can leave the accelerator in an unrecoverable state
    # that clears after one failed attempt; retry to absorb that.
    last_err = None
    for attempt in range(3):
        try:
            res = run_bass_kernel_spmd(nc, in_maps, list(range(NC_CORES)))
            break
        except Exception as err:  # noqa: BLE001
            last_err = err
            import time
            time.sleep(2.0)
    else:
        raise last_err

    out = np.empty((B, E, N, D), dtype=np.float32)
    for e in range(E):
        out[:, e] = res.results[e]["out"].reshape(B, N, D)
    return out

